# revision 7
# baseline (speedup 1.0000x reference)
"""Trainium2 Bass kernel for the ConditionalVAE sampling decoder.

Strategy: tensor-parallel over the GRU gate dimension across 8 NeuronCores.
Each core holds a 384-column slice (128 per gate r/z/n) of every GRU weight
matrix in SBUF, computes its 128-dim slice of each layer's new hidden state,
and the full hidden state is reassembled with an AllGather per layer per step.
The sampling head (FiLM + projection + logits + gumbel-argmax) is replicated
on every core (identical results), so the token feedback loop needs no extra
communication. Gumbel noise is precomputed on host CPU with jax so that
argmax(logits + g) reproduces jax.random.categorical bit-for-bit.
"""

import numpy as np

VOCAB, EMB, PE_DIM, HID, LAT, NLAYERS, BOS = 128, 256, 1280, 1024, 128, 3, 1
NCORES = 8
B = 256            # num_samples (hardcoded per problem spec)
HSL = HID // NCORES          # 128 hidden dims per core per gate
GSL = 3 * HSL                # 384 gate columns per core
KT_H = HID // 128            # 8 K-tiles over HID
KT_E = EMB // 128            # 2 K-tiles over EMB
BT = B // 128                # 2 B-tiles
MM_DT = "float32"            # matmul dtype: "float32" (exact) or "float32r"

_cache = {}


def _np(x):
    return np.asarray(x)


def _build(T):
    """Build the SPMD Bass program for T decode steps. Returns finalized nc."""
    import concourse.bacc as bacc
    import concourse.mybir as mybir
    import concourse.tile as tile

    fp32 = mybir.dt.float32
    mmdt = getattr(mybir.dt, MM_DT)
    u32 = mybir.dt.uint32
    i32 = mybir.dt.int32
    AF = mybir.ActivationFunctionType
    OP = mybir.AluOpType

    nc = bacc.Bacc(num_devices=NCORES)

    def f32v(ap):
        return ap if MM_DT == "float32" else ap.bitcast(fp32)

    def mmv(ap):
        return ap if MM_DT == "float32" else ap.bitcast(mmdt)

    def dp(name, shape, dtype=fp32, out=False):
        return nc.declare_dram_parameter(name, shape, dtype, isOutput=out)

    # ---- DRAM parameters (per-core contents differ only for sliced weights)
    wih0_d = dp("wih0", [EMB, GSL])          # Wih0[:, cols_c]
    wih1_d = dp("wih1", [HID, GSL])
    wih2_d = dp("wih2", [HID, GSL])
    whh0_d = dp("whh0", [HID, GSL])
    whh1_d = dp("whh1", [HID, GSL])
    whh2_d = dp("whh2", [HID, GSL])
    wproj_d = dp("wproj", [HID, EMB])        # full proj_W
    wembT_d = dp("wembT", [EMB, VOCAB])      # emb_W.T
    wemb_d = dp("wemb", [VOCAB, EMB])        # emb_W
    hinit_d = dp("hinit", [HID, B])          # h0.T (full, replicated)
    hinitc_d = dp("hinitc", [HSL, B])        # this core's slice of h0.T
    film_d = dp("film", [128, 2 * (HID // 128)])  # pre-laid [p, 2k+{0,1}] = (1+gamma, beta)
    x0T_d = dp("x0T", [EMB, B])              # x.T at t=0 (BOS embedding bcast)
    gumb_d = dp("gumb", [T, B, VOCAB])       # temperature-scaled gumbel noise
    toks_d = dp("toks", [T, B], dtype=u32, out=True)
    import os as _os
    DBG = bool(int(_os.environ.get("KBG_DEBUG", "0")))
    DBG_T = int(_os.environ.get("KBG_DEBUG_T", "0"))
    if DBG:
        dbg_x_d = dp("dbg_x", [128, KT_E * B], out=True)     # xT sbuf after step0
        dbg_h_d = dp("dbg_h", [NLAYERS, 128, KT_H * B], out=True)  # hT after step0
        dbg_p_d = dp("dbg_p", [NLAYERS, 128, B], out=True)   # hnew local after step0
        dbg_lg_d = dp("dbg_lg", [128, BT * VOCAB], out=True) # logits+gumbel step0

    wih_d = [wih0_d, wih1_d, wih2_d]
    whh_d = [whh0_d, whh1_d, whh2_d]

    with tile.TileContext(nc, num_cores=NCORES) as tc:
        with (
            tc.tile_pool(name="wpool", bufs=1) as wp,
            tc.tile_pool(name="state", bufs=2) as st,
            tc.tile_pool(name="work", bufs=2) as wk,
            tc.tile_pool(name="ps", bufs=2, space="PSUM") as ps,
            tc.tile_pool(name="dram", bufs=4, space="DRAM") as dr,
        ):
            # ================= one-time setup =================
            # weights: [K-tiles on partitions] x [k, cols] on free dim
            def load_w(dram_t, K, F):
                kt = K // 128
                sb = wp.tile([128, kt * F], mmdt, tag=f"w{dram_t.name}")
                nc.sync.dma_start(
                    f32v(sb[:]).rearrange("p (k f) -> p k f", k=kt),
                    dram_t[:].rearrange("(k p) f -> p k f", p=128),
                )
                return sb

            wih_sb = [load_w(wih_d[0], EMB, GSL), load_w(wih_d[1], HID, GSL),
                      load_w(wih_d[2], HID, GSL)]
            whh_sb = [load_w(whh_d[l], HID, GSL) for l in range(NLAYERS)]
            wproj_sb = load_w(wproj_d, HID, EMB)       # [128, 8*256]
            wembT_sb = load_w(wembT_d, EMB, VOCAB)     # [128, 2*128]
            wemb_sb = load_w(wemb_d, VOCAB, EMB)       # [128, 256]

            film_sb = wp.tile([128, 2 * KT_H], fp32, tag="film")  # cols: 2 per kt
            nc.sync.dma_start(film_sb[:], film_d[:])

            ones_sb = wp.tile([1, 128], mmdt, tag="ones")
            nc.vector.memset(f32v(ones_sb[:]), 1.0)
            iota_i = wp.tile([128, 1], i32, tag="iotai")
            nc.gpsimd.iota(iota_i[:], pattern=[[0, 1]], base=0, channel_multiplier=1)
            iota_p = wp.tile([128, 1], fp32, tag="iotap")
            nc.vector.tensor_copy(iota_p[:], iota_i[:])
            # identity for PE transpose
            iden_i = wp.tile([128, 128], i32, tag="ideni")
            nc.gpsimd.iota(iden_i[:], pattern=[[1, 128]], base=0, channel_multiplier=0)
            iden_f = wp.tile([128, 128], fp32, tag="idenf")
            nc.vector.tensor_copy(iden_f[:], iden_i[:])
            ident = wp.tile([128, 128], mmdt, tag="ident")
            nc.vector.tensor_scalar(
                f32v(ident[:]), iden_f[:], iota_p[:], None, op0=OP.is_equal
            )

            # state: h.T per layer [128, 8*256]; local slice hprev [128, 256]
            hT = []
            for l in range(NLAYERS):
                t0 = st.tile([128, KT_H * B], mmdt, tag=f"h{l}")
                nc.sync.dma_start(
                    f32v(t0[:]).rearrange("p (k b) -> p k b", k=KT_H),
                    hinit_d[:].rearrange("(k p) b -> p k b", p=128),
                )
                hT.append(t0)
            hprev = []
            for l in range(NLAYERS):
                t0 = st.tile([128, B], fp32, tag=f"hp{l}")
                nc.sync.dma_start(t0[:], hinitc_d[:])
                hprev.append(t0)
            xT = st.tile([128, KT_E * B], mmdt, tag="xT")
            nc.sync.dma_start(
                f32v(xT[:]).rearrange("p (k b) -> p k b", k=KT_E),
                x0T_d[:].rearrange("(k p) b -> p k b", p=128),
            )

            zero_b = wp.tile([128, 1], fp32, tag="zerob")
            nc.vector.memset(zero_b[:], 0.0)

            # ================= decode loop =================
            for t in range(T):
                # gumbel tile for this step [128, 2*V] (B-tile bt at cols bt*V)
                gt = wk.tile([128, BT * VOCAB], fp32, tag="gt")
                nc.sync.dma_start(
                    gt[:].rearrange("p (bt v) -> p bt v", bt=BT),
                    gumb_d[t].rearrange("(bt p) v -> p bt v", p=128),
                )

                new_hT = []
                for l in range(NLAYERS):
                    # --- PSUM accumulators (1 bank each) ---
                    ps_r = ps.tile([128, B], fp32, tag="r")
                    ps_z = ps.tile([128, B], fp32, tag="z")
                    ps_in = ps.tile([128, B], fp32, tag="in")
                    ps_hn = ps.tile([128, B], fp32, tag="hn")

                    # --- gh: h_prev_full @ Whh_l (emitted first: ready at
                    # step start, fills PE while AllGathers are in flight) ---
                    W = whh_sb[l]
                    h_in = hT[l]
                    for g, pst in ((0, ps_r), (1, ps_z), (2, ps_hn)):
                        for k in range(KT_H):
                            nc.tensor.matmul(
                                pst[:],
                                W[:, k * GSL + g * HSL: k * GSL + (g + 1) * HSL],
                                h_in[:, k * B: (k + 1) * B],
                                start=(k == 0), stop=(g == 2 and k == KT_H - 1),
                                skip_group_check=True,
                            )
                    # --- gi: x @ Wih_l accumulated on top (r, z); i_n separate ---
                    W = wih_sb[l]
                    x_in = xT if l == 0 else hT[l - 1]
                    ktin = KT_E if l == 0 else KT_H
                    for g, pst in ((0, ps_r), (1, ps_z), (2, ps_in)):
                        for k in range(ktin):
                            nc.tensor.matmul(
                                pst[:],
                                W[:, k * GSL + g * HSL: k * GSL + (g + 1) * HSL],
                                x_in[:, k * B: (k + 1) * B],
                                start=(g == 2 and k == 0),
                                stop=(k == ktin - 1),
                                skip_group_check=True,
                            )

                    # --- gates ---
                    r_sb = wk.tile([128, B], fp32, tag="r_sb")
                    z_sb = wk.tile([128, B], fp32, tag="z_sb")
                    nc.scalar.activation(r_sb[:], ps_r[:], AF.Sigmoid, bias=zero_b[:, 0:1])
                    nc.scalar.activation(z_sb[:], ps_z[:], AF.Sigmoid, bias=zero_b[:, 0:1])
                    rhn = wk.tile([128, B], fp32, tag="rhn")
                    nc.vector.tensor_tensor(rhn[:], r_sb[:], ps_hn[:], op=OP.mult)
                    nin = wk.tile([128, B], fp32, tag="nin")
                    nc.vector.tensor_tensor(nin[:], rhn[:], ps_in[:], op=OP.add)
                    n_sb = wk.tile([128, B], fp32, tag="n_sb")
                    nc.scalar.activation(n_sb[:], nin[:], AF.Tanh, bias=zero_b[:, 0:1])
                    # h_new = n + z*(h_prev - n)
                    hmn = wk.tile([128, B], fp32, tag="hmn")
                    nc.vector.tensor_tensor(hmn[:], hprev[l][:], n_sb[:], op=OP.subtract)
                    hz = wk.tile([128, B], fp32, tag="hz")
                    nc.vector.tensor_tensor(hz[:], hmn[:], z_sb[:], op=OP.mult)
                    hnew = st.tile([128, B], fp32, tag=f"hp{l}")
                    nc.vector.tensor_tensor(hnew[:], hz[:], n_sb[:], op=OP.add)
                    hprev[l] = hnew

                    # --- AllGather h_l ---
                    ag_in = dr.tile([HSL, B], fp32, tag="agin")
                    nc.gpsimd.dma_start(ag_in[:], hnew[:])
                    ag_out = dr.tile([HID, B], fp32, tag="agout", addr_space="Shared")
                    nc.gpsimd.collective_compute(
                        "AllGather", OP.bypass,
                        replica_groups=[list(range(NCORES))],
                        ins=[ag_in[:].opt()],
                        outs=[ag_out[:].opt()],
                    )
                    hfull = st.tile([128, KT_H * B], mmdt, tag=f"h{l}")
                    nc.gpsimd.dma_start(
                        f32v(hfull[:]).rearrange("p (k b) -> p k b", k=KT_H),
                        ag_out[:].rearrange("(k p) b -> p k b", p=128),
                    )
                    new_hT.append(hfull)
                    hT[l] = hfull

                # ================= head (replicated on all cores) =================
                # FiLM: out.T = h2.T * (1+gamma) + beta, per K-tile scalars
                outT = wk.tile([128, KT_H * B], mmdt, tag="outT")
                for k in range(KT_H):
                    nc.vector.tensor_scalar(
                        f32v(outT[:])[:, k * B: (k + 1) * B],
                        f32v(hT[2][:])[:, k * B: (k + 1) * B],
                        film_sb[:, 2 * k: 2 * k + 1],
                        film_sb[:, 2 * k + 1: 2 * k + 2],
                        op0=OP.mult, op1=OP.add,
                    )
                # p.T = proj_W.T @ out.T  -> [2 E-tiles, B]
                ps_p0 = ps.tile([128, B], fp32, tag="r")
                ps_p1 = ps.tile([128, B], fp32, tag="z")
                for e, pst in ((0, ps_p0), (1, ps_p1)):
                    for k in range(KT_H):
                        nc.tensor.matmul(
                            pst[:],
                            wproj_sb[:, k * EMB + e * 128: k * EMB + (e + 1) * 128],
                            outT[:, k * B: (k + 1) * B],
                            start=(k == 0), stop=(k == KT_H - 1),
                            skip_group_check=True,
                        )
                pT_sb = wk.tile([128, KT_E * B], mmdt, tag="pT_sb")
                nc.scalar.copy(f32v(pT_sb[:])[:, 0:B], ps_p0[:])
                nc.scalar.copy(f32v(pT_sb[:])[:, B: 2 * B], ps_p1[:])

                # logits[bt] = p @ emb_W.T : lhsT = pT_sb slices, rhs = wembT
                ps_lg0 = ps.tile([128, VOCAB], fp32, tag="in")
                ps_lg1 = ps.tile([128, VOCAB], fp32, tag="hn")
                for bt, pst in ((0, ps_lg0), (1, ps_lg1)):
                    for e in range(KT_E):
                        nc.tensor.matmul(
                            pst[:],
                            pT_sb[:, e * B + bt * 128: e * B + (bt + 1) * 128],
                            wembT_sb[:, e * VOCAB: (e + 1) * VOCAB],
                            start=(e == 0), stop=(e == KT_E - 1),
                            skip_group_check=True,
                        )
                # + gumbel  (temperature folded into gumbel on host)
                lg = wk.tile([128, BT * VOCAB], fp32, tag="lg")
                nc.vector.tensor_tensor(
                    lg[:, 0:VOCAB], ps_lg0[:], gt[:, 0:VOCAB], op=OP.add)
                nc.vector.tensor_tensor(
                    lg[:, VOCAB: 2 * VOCAB], ps_lg1[:], gt[:, VOCAB: 2 * VOCAB], op=OP.add)

                # argmax over vocab per row; first-index tie-break matches jnp.argmax
                m8 = wk.tile([128, BT * 8], fp32, tag="m8")
                idx8 = wk.tile([128, BT * 8], u32, tag="idx8")
                idxf = wk.tile([128, BT], fp32, tag="idxf")
                for bt in range(BT):
                    nc.vector.max(m8[:, bt * 8: (bt + 1) * 8], lg[:, bt * VOCAB: (bt + 1) * VOCAB])
                    nc.vector.max_index(
                        idx8[:, bt * 8: (bt + 1) * 8],
                        m8[:, bt * 8: (bt + 1) * 8],
                        lg[:, bt * VOCAB: (bt + 1) * VOCAB],
                    )
                    # token out
                    nc.sync.dma_start(
                        toks_d[t, bt * 128: (bt + 1) * 128].rearrange("(p c) -> p c", c=1),
                        idx8[:, bt * 8: bt * 8 + 1],
                    )
                    nc.vector.tensor_copy(idxf[:, bt: bt + 1], idx8[:, bt * 8: bt * 8 + 1])

                # idx row [1, 256]: PE transpose of idx columns
                ps_tr = ps.tile([128, B], fp32, tag="r")
                for bt in range(BT):
                    nc.tensor.transpose(
                        ps_tr[0:1, bt * 128: (bt + 1) * 128],
                        mmv(idxf[:, bt: bt + 1]), ident[:])
                idxrow = wk.tile([1, B], mmdt, tag="idxrow")
                nc.vector.tensor_copy(f32v(idxrow[:]), ps_tr[0:1, 0:B])

                # broadcast idx across partitions: ones.T @ idxrow
                ps_bc = ps.tile([128, B], fp32, tag="z")
                nc.tensor.matmul(ps_bc[:], ones_sb[:], idxrow[:], start=True, stop=True,
                                 skip_group_check=True)
                # onehot.T [V, B] = (bcast == iota_p)
                oh = wk.tile([128, B], mmdt, tag="oh")
                nc.vector.tensor_scalar(
                    f32v(oh[:]), ps_bc[:], iota_p[:], None, op0=OP.is_equal)

                # x.T = emb_W.T @ onehot  -> [2 E-tiles, B]
                ps_x0 = ps.tile([128, B], fp32, tag="in")
                ps_x1 = ps.tile([128, B], fp32, tag="hn")
                for e, pst in ((0, ps_x0), (1, ps_x1)):
                    nc.tensor.matmul(
                        pst[:], wemb_sb[:, e * 128: (e + 1) * 128], oh[:],
                        start=True, stop=True, skip_group_check=True)
                xT = st.tile([128, KT_E * B], mmdt, tag="xT")
                nc.scalar.copy(f32v(xT[:])[:, 0:B], ps_x0[:])
                nc.scalar.copy(f32v(xT[:])[:, B: 2 * B], ps_x1[:])

                if DBG and t == DBG_T:
                    nc.sync.dma_start(dbg_x_d[:], f32v(xT[:]))
                    for l in range(NLAYERS):
                        nc.sync.dma_start(dbg_h_d[l], f32v(hT[l][:]))
                        nc.sync.dma_start(dbg_p_d[l], hprev[l][:])
                    nc.sync.dma_start(dbg_lg_d[:], lg[:])

    nc.finalize()
    return nc


def _host_prep(inputs):
    """Host-side precompute. Returns dict of per-name full arrays + T."""
    import jax
    import jax.numpy as jnp

    f32 = np.float32
    num_samples = int(_np(inputs["num_samples"]))
    T = int(_np(inputs["max_len"]))
    assert num_samples == B, f"kernel hardcoded for num_samples={B}, got {num_samples}"

    pe = _np(inputs["protein_embedding"]).astype(f32).reshape(1, PE_DIM)
    z = _np(inputs["z"]).astype(f32)
    init_W = _np(inputs["init_W"]).astype(f32)
    init_b = _np(inputs["init_b"]).astype(f32)
    peB = np.broadcast_to(pe, (B, PE_DIM)).astype(f32)
    h0 = np.maximum(np.concatenate([z, peB], 1) @ init_W + init_b, 0.0).astype(f32)

    gamma = (pe[0] @ _np(inputs["gamma_W"]).astype(f32) + _np(inputs["gamma_b"]).astype(f32)).astype(f32)
    beta = (pe[0] @ _np(inputs["beta_W"]).astype(f32) + _np(inputs["beta_b"]).astype(f32)).astype(f32)
    fg = (1.0 + gamma).reshape(HID // 128, 128).T
    fb = beta.reshape(HID // 128, 128).T
    film = np.empty((128, 2 * (HID // 128)), f32)
    film[:, 0::2] = fg
    film[:, 1::2] = fb

    emb_W = _np(inputs["emb_W"]).astype(f32)
    proj_W = _np(inputs["proj_W"]).astype(f32)
    Wih = [_np(w).astype(f32) for w in inputs["gru_Wih"]]
    Whh = [_np(w).astype(f32) for w in inputs["gru_Whh"]]
    bih = [_np(b).astype(f32) for b in inputs["gru_bih"]]
    bhh = [_np(b).astype(f32) for b in inputs["gru_bhh"]]
    for b_ in bih + bhh:
        assert not np.any(b_), "kernel assumes zero GRU biases (as in setup_inputs)"

    temp = float(max(1e-8, float(_np(inputs["temperature"]).reshape(-1)[0])))

    # gumbel noise, exactly as jax.random.categorical draws it (on CPU)
    cpu = jax.devices("cpu")[0]
    key = inputs["sample_key"]
    if not isinstance(key, jax.Array) or not jnp.issubdtype(key.dtype, jax.dtypes.prng_key):
        kd = _np(key)
        impl = "rbg" if kd.size == 4 else "threefry2x32"
        key = jax.random.wrap_key_data(jnp.asarray(kd, jnp.uint32), impl=impl)
    key = jax.device_put(key, cpu)

    # NOTE: must draw per-key (no vmap) — the rbg PRNG is not vmap-invariant,
    # and the reference applies categorical(keys[t], .) per scan step.
    _split = jax.jit(lambda k: jax.random.split(k, T), backend="cpu")
    _gone = jax.jit(lambda kk: jax.random.gumbel(kk, (B, VOCAB), jnp.float32),
                    backend="cpu")
    ks = _split(key)
    G = np.stack([np.asarray(_gone(ks[i])) for i in range(T)]).astype(f32)
    G = (G * f32(temp)).astype(f32)   # argmax(l/temp + g) == argmax(l + temp*g)

    x0T = np.broadcast_to(emb_W[BOS][:, None], (EMB, B)).astype(f32).copy()

    return dict(T=T, h0=h0, film=film, emb_W=emb_W, proj_W=proj_W,
                Wih=Wih, Whh=Whh, G=G, x0T=x0T)


def kernel(**inputs):
    from concourse.bass_utils import run_bass_kernel_spmd

    p = _host_prep(inputs)
    T = p["T"]

    if T not in _cache:
        _cache[T] = _build(T)
    nc = _cache[T]

    f32 = np.float32
    h0T = np.ascontiguousarray(p["h0"].T)                # [HID, B]
    in_maps = []
    for c in range(NCORES):
        cols = np.concatenate([np.arange(g * HID + c * HSL, g * HID + (c + 1) * HSL)
                               for g in range(3)])
        m = {
            "wih0": np.ascontiguousarray(p["Wih"][0][:, cols]),
            "wih1": np.ascontiguousarray(p["Wih"][1][:, cols]),
            "wih2": np.ascontiguousarray(p["Wih"][2][:, cols]),
            "whh0": np.ascontiguousarray(p["Whh"][0][:, cols]),
            "whh1": np.ascontiguousarray(p["Whh"][1][:, cols]),
            "whh2": np.ascontiguousarray(p["Whh"][2][:, cols]),
            "wproj": p["proj_W"],
            "wembT": np.ascontiguousarray(p["emb_W"].T),
            "wemb": p["emb_W"],
            "hinit": h0T,
            "hinitc": np.ascontiguousarray(h0T[c * HSL:(c + 1) * HSL]),
            "film": p["film"],
            "x0T": p["x0T"],
            "gumb": p["G"],
        }
        in_maps.append({k: np.ascontiguousarray(v, dtype=f32) for k, v in m.items()})

    res = run_bass_kernel_spmd(nc, in_maps, list(range(NCORES)))
    toks = res.results[0]["toks"].astype(np.int64).astype(np.int32)  # [T, B]
    seq = np.concatenate([np.full((1, B), BOS, np.int32), toks], axis=0).T
    return np.ascontiguousarray(seq)


def profile_once(inputs):
    """Run once with NTFF tracing; return exec_time_ns (or None)."""
    from concourse.bass_utils import run_bass_kernel_spmd
    p = _host_prep(inputs)
    T = p["T"]
    nc = _cache.get(T) or _build(T)
    _cache[T] = nc
    f32 = np.float32
    h0T = np.ascontiguousarray(p["h0"].T)
    in_maps = []
    for c in range(NCORES):
        cols = np.concatenate([np.arange(g * HID + c * HSL, g * HID + (c + 1) * HSL)
                               for g in range(3)])
        m = {
            "wih0": p["Wih"][0][:, cols], "wih1": p["Wih"][1][:, cols],
            "wih2": p["Wih"][2][:, cols], "whh0": p["Whh"][0][:, cols],
            "whh1": p["Whh"][1][:, cols], "whh2": p["Whh"][2][:, cols],
            "wproj": p["proj_W"], "wembT": p["emb_W"].T, "wemb": p["emb_W"],
            "hinit": h0T, "hinitc": h0T[c * HSL:(c + 1) * HSL],
            "film": p["film"], "x0T": p["x0T"], "gumb": p["G"],
        }
        in_maps.append({k: np.ascontiguousarray(v, dtype=f32) for k, v in m.items()})
    import tempfile
    td = tempfile.mkdtemp(prefix="ktrace_")
    try:
        res = run_bass_kernel_spmd(nc, in_maps, list(range(NCORES)), trace=True, tmpdir=td)
        print("trace dir:", td)
        return res.exec_time_ns
    except Exception as e:
        print("trace failed:", e)
        return None
